# revision 1
# baseline (speedup 1.0000x reference)
"""Trainium2 Bass kernel for MinimalThinkingRefiner.

out = where(mask==2, x + alpha*(x*scale + shift), x)
    = x * (1 + t*alpha*scale) + t*alpha*shift,   t = (mask==2) per row

Data-parallel across 8 cores: rows of the flattened [16384, 4096] tensor are
split into 8 contiguous shards of 2048 rows. Per core, 16 tiles of
[128 rows, 4096] f32 (2MB each). Per tile:
  ACT : C   = Identity(scale_rep * t_alpha[p] + 1)      (per-partition scalar)
  DVE : xc  = x * C                    (tensor_tensor, in-place into x tile)
  DVE : out = (shift_rep * t_alpha[p]) + xc   (fused scalar_tensor_tensor)
t_alpha[p] = alpha if mask[row]==2 else 0 is per-partition, so it rides in the
scalar slot; scale/shift vary along the free axis and are partition-broadcast
once at startup.
"""

import sys

if "/opt/trn_rl_repo" not in sys.path:
    sys.path.insert(0, "/opt/trn_rl_repo")

import numpy as np

import concourse.bacc as bacc
import concourse.bass as bass
import concourse.mybir as mybir
import concourse.tile as tile
from concourse.bass_utils import run_bass_kernel_spmd

N_CORES = 8
B, S, H = 4, 4096, 4096
ROWS = B * S            # 16384
RPC = ROWS // N_CORES   # 2048 rows per core
P = 128
NT = RPC // P           # 16 tiles per core

_cached = {}


def build_nc():
    nc = bacc.Bacc("TRN2", debug=False, target_bir_lowering=False)

    x = nc.dram_tensor("x", [RPC, H], mybir.dt.float32, kind="ExternalInput")
    mask = nc.dram_tensor("mask", [RPC], mybir.dt.int32, kind="ExternalInput")
    scale = nc.dram_tensor("scale", [H], mybir.dt.float32, kind="ExternalInput")
    shift = nc.dram_tensor("shift", [H], mybir.dt.float32, kind="ExternalInput")
    alpha = nc.dram_tensor("alpha", [1], mybir.dt.float32, kind="ExternalInput")
    out = nc.dram_tensor("out", [RPC, H], mybir.dt.float32, kind="ExternalOutput")

    fp32 = mybir.dt.float32

    with tile.TileContext(nc) as tc:
        with (
            tc.tile_pool(name="const", bufs=1) as cpool,
            tc.tile_pool(name="xbuf", bufs=4) as xpool,
            tc.tile_pool(name="cbuf", bufs=3) as cbufpool,
        ):
            sc_row = cpool.tile([1, H], fp32)
            nc.sync.dma_start(sc_row[:], scale[None, :])
            sh_row = cpool.tile([1, H], fp32)
            nc.sync.dma_start(sh_row[:], shift[None, :])
            al_row = cpool.tile([1, 1], fp32)
            nc.sync.dma_start(al_row[:], alpha[None, :])
            m_t = cpool.tile([P, NT], mybir.dt.int32)
            nc.sync.dma_start(m_t[:], mask.rearrange("(n p) -> p n", p=P))

            sc_rep = cpool.tile([P, H], fp32)
            nc.gpsimd.partition_broadcast(sc_rep[:], sc_row[0:1, :])
            sh_rep = cpool.tile([P, H], fp32)
            nc.gpsimd.partition_broadcast(sh_rep[:], sh_row[0:1, :])
            al_rep = cpool.tile([P, 1], fp32)
            nc.gpsimd.partition_broadcast(al_rep[:], al_row[0:1, :])

            # t_alpha[p, i] = alpha if mask[i*128+p] == 2 else 0
            t_alpha = cpool.tile([P, NT], fp32)
            nc.vector.tensor_scalar(
                t_alpha[:], m_t[:], 2, al_rep[:],
                op0=mybir.AluOpType.is_equal, op1=mybir.AluOpType.mult,
            )

            for i in range(NT):
                xt = xpool.tile([P, H], fp32)
                nc.sync.dma_start(xt[:], x[bass.ts(i, P), :])

                ct = cbufpool.tile([P, H], fp32)
                # C = scale_rep * t_alpha + 1
                nc.scalar.activation(
                    ct[:], sc_rep[:], mybir.ActivationFunctionType.Identity,
                    bias=1.0, scale=t_alpha[:, i : i + 1],
                )
                # xt = x * C
                nc.vector.tensor_mul(xt[:], xt[:], ct[:])
                # xt = (shift_rep * t_alpha) + xt
                nc.vector.scalar_tensor_tensor(
                    xt[:], sh_rep[:], t_alpha[:, i : i + 1], xt[:],
                    op0=mybir.AluOpType.mult, op1=mybir.AluOpType.add,
                )
                nc.sync.dma_start(out[bass.ts(i, P), :], xt[:])

    nc.compile()
    return nc


def kernel(**inputs) -> np.ndarray:
    x = np.ascontiguousarray(np.asarray(inputs["hidden_states"], dtype=np.float32)).reshape(ROWS, H)
    mask = np.ascontiguousarray(np.asarray(inputs["input_mask"], dtype=np.int32)).reshape(ROWS)
    scale = np.ascontiguousarray(np.asarray(inputs["scale"], dtype=np.float32))
    shift = np.ascontiguousarray(np.asarray(inputs["shift"], dtype=np.float32))
    alpha = np.asarray(inputs["alpha"], dtype=np.float32).reshape(1)

    if "nc" not in _cached:
        _cached["nc"] = build_nc()
    nc = _cached["nc"]

    in_maps = []
    for c in range(N_CORES):
        sl = slice(c * RPC, (c + 1) * RPC)
        in_maps.append({
            "x": x[sl],
            "mask": mask[sl],
            "scale": scale,
            "shift": shift,
            "alpha": alpha,
        })

    res = run_bass_kernel_spmd(nc, in_maps, core_ids=list(range(N_CORES)))
    out = np.concatenate([res.results[c]["out"] for c in range(N_CORES)], axis=0)
    return out.reshape(B, S, H)
